# revision 10
# baseline (speedup 1.0000x reference)
"""Multi-head self-attention (B=2, N=2048, C=1024, H=16) on 8 trn2 NeuronCores.

Sharding: core = b * 4 + g  (data parallel over batch B=2, tensor parallel
over 4 head-groups of 4 heads each).  Each core computes its head-group's
QKV projections, attention, and a partial output projection; the host sums
the 4 partials per batch (the "all-reduce") and adds the bias.

All matmul operands are bf16 with fp32 PSUM accumulation (fp8 anywhere in
the attention math exceeds the 2e-2 error budget: softmax rows are peaked,
so score/V errors do not average away).  The two performance levers over a
phase-serial emission are:

  - K=64 score matmuls are emitted as head pairs at PE base partitions
    0/64: distinct row-groups execute concurrently on the 16-subarray PE
    (measured ~2x for row-tiled small-K matmuls), halving score cost.
    exp tiles hold exactly one pair so no pair straddles a PSUM-ring wait.
  - A two-pass window pipeline: for each (head-pair, 512-query-window),
    pass A streams scores+exp (ScalarE, the ~128us roofline of the kernel);
    pass B (AV via the [V|ones] row-sum trick, then normalize) and filler
    work (V/QK projections, output-projection slices) are interleaved into
    the NEXT window's pass A from a pending queue, keeping both PE and ACT
    streams dense (PE gaps >3us re-throttle the PE clock).

PSUM budget: score/projection ring 3x[128,1024] (6 banks) + att0 + att1
(2 banks); output-projection slices reuse the att banks between AV rounds.
"""

import sys
from collections import deque

for _p in ("/opt/trn_rl_repo",):
    if _p not in sys.path:
        sys.path.append(_p)

import numpy as np

import concourse.bass as bass
import concourse.mybir as mybir
import concourse.tile as tile
from concourse import bacc
from concourse.bass_utils import run_bass_kernel_spmd

B, N, C = 2, 2048, 1024
H = 16
HS = C // H  # 64
G = 4  # head groups (tensor-parallel factor)
HPG = H // G  # heads per group = 4
GC = HPG * HS  # channels per group = 256
SCALE = HS**-0.5
P = 128
MC = N // P  # 16 key chunks
KC = C // P  # 8 contraction chunks for the qkv projection
F32 = mybir.dt.float32
BF16 = mybir.dt.bfloat16
Exp = mybir.ActivationFunctionType.Exp

_CACHED = {}


def build_bass(loop_n=1, stage_in_loop=True, pb_rate=2, slab2=True, unpair=False):
    nc = bacc.Bacc("TRN2", target_bir_lowering=False, debug=False)
    xT = nc.dram_tensor("xT", (C, N), BF16, kind="ExternalInput").ap()
    wqT = nc.dram_tensor("wqT", (C, GC), BF16, kind="ExternalInput").ap()
    wkT = nc.dram_tensor("wkT", (C, GC), BF16, kind="ExternalInput").ap()
    wvT = nc.dram_tensor("wvT", (C, GC), BF16, kind="ExternalInput").ap()
    woT = nc.dram_tensor("woT", (GC, C), BF16, kind="ExternalInput").ap()
    outT = nc.dram_tensor("outT", (C, N), F32, kind="ExternalOutput").ap()

    # PSUM score-ring geometry: exp reads tiles of `sw` fp32 (sn slabs of
    # 512); 3 bufs of 1024 or 2 bufs of 1536 both fill 6 banks.
    sw, sbufs = (1024, 3) if slab2 else (1536, 2)

    with tile.TileContext(nc) as tc:
        import contextlib

        ctx = contextlib.ExitStack()
        with ctx:
            wpool = ctx.enter_context(tc.tile_pool(name="wpool", bufs=1))
            mpool = ctx.enter_context(tc.tile_pool(name="mpool", bufs=1))
            psum = ctx.enter_context(tc.tile_pool(name="psum", bufs=1, space="PSUM"))
            opool = ctx.enter_context(tc.tile_pool(name="opool", bufs=4))

            # ---- persistent tiles ------------------------------------------
            xr = mpool.tile([P, KC, N], BF16)  # x^T
            wqr = wpool.tile([P, KC, GC], BF16)
            wkr = wpool.tile([P, KC, GC], BF16)
            wvr = wpool.tile([P, KC, GC], BF16)
            wor = wpool.tile([P, GC // P, C], BF16)
            qr = mpool.tile([P, 2, N], BF16)  # Q^T; dim1 = head pair (mch)
            kr = mpool.tile([P, 2, N], BF16)
            # va blocks: even heads [V | ones], odd heads [ones | V] so the
            # attention output lands on the partition half matching the
            # head's slot in `an` (the ones columns give the softmax row
            # sums for free: PE matmul cost only depends on the moving dim).
            va = mpool.tile([P, MC, HPG, P], BF16)
            an = mpool.tile([P, 2, N], BF16)  # normalized attn^T

            def prologue():
                nc.gpsimd.memset(va[:, :, 0::2, HS:P], 1.0)
                nc.gpsimd.memset(va[:, :, 1::2, 0:HS], 1.0)

            # ---- input loads (2 DGE queues, ACT kept free) -----------------
            def stage_all():
                x3 = xT.rearrange("(c p) n -> p c n", p=P)
                nc.sync.dma_start(
                    out=wkr[:], in_=wkT.rearrange("(c p) m -> p c m", p=P)
                )
                nc.gpsimd.dma_start(
                    out=wqr[:], in_=wqT.rearrange("(c p) m -> p c m", p=P)
                )
                engs = [nc.sync, nc.gpsimd]
                for j in range(4):
                    engs[j % 2].dma_start(
                        out=xr[:, 2 * j : 2 * j + 2, :], in_=x3[:, 2 * j : 2 * j + 2, :]
                    )
                nc.gpsimd.dma_start(
                    out=wvr[:], in_=wvT.rearrange("(c p) m -> p c m", p=P)
                )
                nc.sync.dma_start(
                    out=wor[:], in_=woT.rearrange("(c p) o -> p c o", p=P)
                )

            # ---- QKV projections (single 512-query/key chunk per call) -----
            def qk_proj_nch(w_r, dst, mch, nch):
                acc = psum.tile([P, sw], F32, tag="s", bufs=sbufs, name="acc")
                for k in range(KC):
                    nc.tensor.matmul(
                        acc[:, 0:512],
                        w_r[:, k, mch * P : (mch + 1) * P],
                        xr[:, k, nch * 512 : (nch + 1) * 512],
                        start=(k == 0),
                        stop=(k == KC - 1),
                    )
                nc.vector.tensor_copy(
                    dst[:, mch, nch * 512 : (nch + 1) * 512], acc[:, 0:512]
                )

            def v_proj_quarter(mq):
                for m in range(4 * mq, 4 * mq + 4):
                    vacc = psum.tile([P, sw], F32, tag="s", bufs=sbufs, name="vacc")
                    for k in range(KC):
                        nc.tensor.matmul(
                            vacc[:, 0:GC],
                            xr[:, k, m * P : (m + 1) * P],
                            wvr[:, k, :],
                            start=(k == 0),
                            stop=(k == KC - 1),
                        )
                    vh = vacc[:, 0:GC].rearrange("p (h e) -> p h e", h=HPG)
                    nc.vector.tensor_copy(va[:, m, 0::2, 0:HS], vh[:, 0::2, :])
                    nc.vector.tensor_copy(va[:, m, 1::2, HS:P], vh[:, 1::2, :])

            # ---- attention pass A: scores + exp stream for one window ------
            # Head pair slabs (par 0/1, bases 0/64) are emitted back-to-back:
            # distinct PE row groups run concurrently.  AV for THIS window is
            # interleaved two exp-tiles behind (its inputs are then ready, so
            # the in-order PE never stalls on ACT).  Queued work (projection
            # chunks, previous window's output slices) drains fast in the
            # first tiles -- o_ps slices MUST be emitted before av(0) takes
            # the att banks or the ring order deadlocks -- then slowly.
            def pass_a(hp, q, pw, pending, avs, norm):
                qsl = slice(q * 512, (q + 1) * 512)
                n_slabs = 2 * MC  # 32 x [128, 512]
                ti = 0
                j = 0
                while j < n_slabs:
                    n_sl = min(sw // 512, n_slabs - j)
                    st = psum.tile([P, sw], F32, tag="s", bufs=sbufs, name="st")
                    for jj in range(n_sl):
                        m, par = (j + jj) // 2, (j + jj) % 2
                        off = 0 if unpair else 64 * par  # unpair: timing probe only
                        nc.tensor.matmul(
                            st[:, jj * 512 : (jj + 1) * 512],
                            kr[off : off + 64, hp, m * P : (m + 1) * P],
                            qr[off : off + 64, hp, qsl],
                            start=True,
                            stop=True,
                        )
                    nc.scalar.activation(
                        pw[:, j * 512 : (j + n_sl) * 512],
                        st[:, 0 : n_sl * 512],
                        Exp,
                        scale=SCALE,
                    )
                    j += n_sl
                    for _ in range(4 if ti < 3 else pb_rate):
                        if pending:
                            pending.popleft()()
                    if ti >= 2:
                        avs[ti - 2]()
                    ti += 1
                avs[-2]()
                avs[-1]()
                norm()

            # ---- attention pass B: AV + normalize for one window -----------
            def make_pb(hp, q, pw):
                qsl = slice(q * 512, (q + 1) * 512)
                att = {}
                pv = pw.rearrange("p (m x) -> p m x", x=1024)

                def av(m):
                    def f():
                        if "a0" not in att:
                            att["a0"] = psum.tile(
                                [P, 512], F32, tag="att0", bufs=1, name="att0"
                            )
                            att["a1"] = psum.tile(
                                [P, 512], F32, tag="att1", bufs=1, name="att1"
                            )
                        for par, at in ((0, att["a0"]), (1, att["a1"])):
                            nc.tensor.matmul(
                                at[:],
                                va[:, m, 2 * hp + par, :],
                                pv[:, m, par * 512 : (par + 1) * 512],
                                start=(m == 0),
                                stop=(m == MC - 1),
                            )

                    return f

                def norm():
                    # Even head: attn rows 0:64, rowsum 64:128; odd head
                    # flipped (va block order).  The custom recip uop only
                    # works at base partition 0; cross-partition moves go
                    # through SBUF->SBUF DMA.
                    au0 = mpool.tile([P, 512], F32, tag="au", bufs=4, name="au0")
                    au1 = mpool.tile([P, 512], F32, tag="au", bufs=4, name="au1")
                    rr0 = mpool.tile([P, 512], F32, tag="rr", bufs=4, name="rr0")
                    rr1 = mpool.tile([P, 512], F32, tag="rr", bufs=4, name="rr1")
                    nc.vector.tensor_copy(au0[:], att["a0"][:])
                    nc.vector.tensor_copy(au1[:], att["a1"][:])
                    nc.sync.dma_start(out=rr0[0:64, :], in_=au0[64:128, :])
                    nc.vector.reciprocal_approx_fast(rr0[0:64, :], rr0[0:64, :])
                    nc.vector.tensor_mul(an[0:64, hp, qsl], au0[0:64, :], rr0[0:64, :])
                    nc.vector.reciprocal_approx_fast(rr1[0:64, :], au1[0:64, :])
                    nc.gpsimd.dma_start(out=rr1[64:128, :], in_=rr1[0:64, :])
                    nc.vector.tensor_mul(
                        an[64:128, hp, qsl], au1[64:128, :], rr1[64:128, :]
                    )

                return [av(m) for m in range(MC)], norm

            # ---- output projection, one 128-channel slice at a time --------
            def out_step(q, och):
                def f():
                    o_ps = psum.tile(
                        [P, 512], F32, tag="att0" if och % 2 == 0 else "att1",
                        bufs=1, name="o_ps",
                    )
                    for c in range(GC // P):
                        nc.tensor.matmul(
                            o_ps[:],
                            wor[:, c, och * P : (och + 1) * P],
                            an[:, c, q * 512 : (q + 1) * 512],
                            start=(c == 0),
                            stop=(c == GC // P - 1),
                        )
                    o_sb = opool.tile([P, 512], F32, name="o_sb")
                    nc.vector.tensor_copy(o_sb[:], o_ps[:])
                    eng = nc.sync if och % 2 == 0 else nc.gpsimd
                    eng.dma_start(
                        out=outT[och * P : (och + 1) * P, q * 512 : (q + 1) * 512],
                        in_=o_sb,
                    )

                return f

            # ---- body: windows with pipelined filler work ------------------
            # Window (hp, q) only reads qr chunk nch=q, so Q chunks are
            # computed one window ahead; K chunks for pair 1 spread over the
            # pair-0 windows.  Per-window PE load stays under the ~16.6us
            # exp stream.
            def body(staged):
                prologue()
                if staged:
                    stage_all()
                for nch in range(4):
                    qk_proj_nch(wkr, kr, 0, nch)
                qk_proj_nch(wqr, qr, 0, 0)
                for mq in range(3):
                    v_proj_quarter(mq)

                def qk(w_r, dst, mch, nch):
                    return lambda: qk_proj_nch(w_r, dst, mch, nch)

                filler = {
                    (0, 0): [lambda: v_proj_quarter(3), qk(wqr, qr, 0, 1)],
                    (0, 1): [qk(wkr, kr, 1, 0), qk(wkr, kr, 1, 1), qk(wqr, qr, 0, 2)],
                    (0, 2): [qk(wkr, kr, 1, 2), qk(wkr, kr, 1, 3), qk(wqr, qr, 0, 3)],
                    (0, 3): [qk(wqr, qr, 1, 0), qk(wqr, qr, 1, 1)],
                    (1, 0): [qk(wqr, qr, 1, 2)],
                    (1, 1): [qk(wqr, qr, 1, 3)],
                }
                pending = deque()
                for hp in range(2):
                    for q in range(4):
                        pw = mpool.tile(
                            [P, MC * 1024], BF16, tag="pw", bufs=2, name="pw"
                        )
                        pending.extend(filler.get((hp, q), []))
                        avs, norm = make_pb(hp, q, pw)
                        pass_a(hp, q, pw, pending, avs, norm)
                        if hp == 1:
                            pending.extend(out_step(q, och) for och in range(C // P))
                while pending:
                    pending.popleft()()

            if loop_n > 1:
                if not stage_in_loop:
                    stage_all()
                ET = mybir.EngineType
                with tc.For_i(
                    0,
                    loop_n,
                    1,
                    hint_engines=(ET.PE, ET.Activation, ET.DVE, ET.SP),
                ):
                    body(staged=stage_in_loop)
            else:
                body(staged=True)

    nc.compile()
    return nc


def shard_inputs(x, w_qkv, w_out):
    """Host-side shard prep. Returns in_maps for cores 0..7 (core = b*4+g).

    All inputs ship as bf16 (the PE consumes bf16 directly); accumulation
    on chip is fp32 and the output returns fp32."""
    import ml_dtypes

    bf16 = ml_dtypes.bfloat16
    # w_qkv row d = c_idx*3 + t  (t: 0=q, 1=k, 2=v)  [stride-3 interleave]
    wr = np.ascontiguousarray(w_qkv.reshape(C, 3, C))
    in_maps = []
    for b in range(B):
        xTb = np.ascontiguousarray(x[b].T.astype(bf16))
        for g in range(G):
            sl = slice(g * GC, (g + 1) * GC)
            in_maps.append(
                {
                    "xT": xTb,
                    "wqT": np.ascontiguousarray(wr[sl, 0, :].T.astype(bf16)),
                    "wkT": np.ascontiguousarray(wr[sl, 1, :].T.astype(bf16)),
                    "wvT": np.ascontiguousarray(wr[sl, 2, :].T.astype(bf16)),
                    "woT": np.ascontiguousarray(w_out[:, sl].T.astype(bf16)),
                }
            )
    return in_maps


def kernel(x, w_qkv, w_out, b_out):
    x = np.asarray(x, dtype=np.float32)
    w_qkv = np.asarray(w_qkv, dtype=np.float32)
    w_out = np.asarray(w_out, dtype=np.float32)
    b_out = np.asarray(b_out, dtype=np.float32)

    if "nc" not in _CACHED:
        _CACHED["nc"] = build_bass()
    nc = _CACHED["nc"]

    in_maps = shard_inputs(x, w_qkv, w_out)
    res = run_bass_kernel_spmd(nc, in_maps, core_ids=list(range(8)))

    out = np.empty((B, N, C), dtype=np.float32)
    for b in range(B):
        acc = res.results[b * G + 0]["outT"].astype(np.float32)
        for g in range(1, G):
            acc = acc + res.results[b * G + g]["outT"]
        out[b] = acc.T + b_out
    return out


if __name__ == "__main__":
    rng = np.random.default_rng(0)
    x = rng.standard_normal((B, N, C), dtype=np.float32)
    w_qkv = rng.standard_normal((3 * C, C), dtype=np.float32) * C**-0.5
    w_out = rng.standard_normal((C, C), dtype=np.float32) * C**-0.5
    b_out = np.zeros((C,), dtype=np.float32)
    got = kernel(x, w_qkv, w_out, b_out)
    print("kernel ran, output shape", got.shape)


# revision 17
# speedup vs baseline: 1.1444x; 1.1444x over previous
"""Multi-head self-attention (B=2, N=2048, C=1024, H=16) on 8 trn2 NeuronCores.

Sharding: core = b * 4 + g  (data parallel over batch B=2, tensor parallel
over 4 head-groups of 4 heads each).  Each core computes its head-group's
QKV projections, attention, and a partial output projection; the host sums
the 4 partials per batch (the "all-reduce") and adds the bias.

All matmul operands are bf16 with fp32 PSUM accumulation (fp8 anywhere in
the attention math exceeds the 2e-2 error budget: softmax rows are peaked,
so score/V errors do not average away).  The two performance levers over a
phase-serial emission are:

  - K=64 score matmuls are emitted as head pairs at PE base partitions
    0/64: distinct row-groups execute concurrently on the 16-subarray PE
    (measured ~2x for row-tiled small-K matmuls), halving score cost.
    exp tiles hold exactly one pair so no pair straddles a PSUM-ring wait.
  - A two-pass window pipeline: for each (head-pair, 512-query-window),
    pass A streams scores+exp (ScalarE, the ~128us roofline of the kernel);
    pass B (AV via the [V|ones] row-sum trick, then normalize) and filler
    work (V/QK projections, output-projection slices) are interleaved into
    the NEXT window's pass A from a pending queue, keeping both PE and ACT
    streams dense (PE gaps >3us re-throttle the PE clock).

PSUM budget: score/projection ring 3x[128,1024] (6 banks) + att0 + att1
(2 banks); output-projection slices reuse the att banks between AV rounds.
"""

import sys
from collections import deque

for _p in ("/opt/trn_rl_repo",):
    if _p not in sys.path:
        sys.path.append(_p)

import numpy as np

import concourse.bass as bass
import concourse.mybir as mybir
import concourse.tile as tile
from concourse import bacc
from concourse.bass_utils import run_bass_kernel_spmd

B, N, C = 2, 2048, 1024
H = 16
HS = C // H  # 64
G = 4  # head groups (tensor-parallel factor)
HPG = H // G  # heads per group = 4
GC = HPG * HS  # channels per group = 256
SCALE = HS**-0.5
P = 128
MC = N // P  # 16 key chunks
KC = C // P  # 8 contraction chunks for the qkv projection
F32 = mybir.dt.float32
BF16 = mybir.dt.bfloat16
Exp = mybir.ActivationFunctionType.Exp

_CACHED = {}


def build_bass(loop_n=1, stage_in_loop=True, pb_rate=2, slab2=True, unpair=False):
    nc = bacc.Bacc("TRN2", target_bir_lowering=False, debug=False)
    xT = nc.dram_tensor("xT", (C, N), BF16, kind="ExternalInput").ap()
    wqT = nc.dram_tensor("wqT", (C, GC), BF16, kind="ExternalInput").ap()
    wkT = nc.dram_tensor("wkT", (C, GC), BF16, kind="ExternalInput").ap()
    wvT = nc.dram_tensor("wvT", (C, GC), BF16, kind="ExternalInput").ap()
    woT = nc.dram_tensor("woT", (GC, C), BF16, kind="ExternalInput").ap()
    outT = nc.dram_tensor("outT", (C, N), F32, kind="ExternalOutput").ap()

    # PSUM score-ring geometry: exp reads tiles of `sw` fp32 (sn slabs of
    # 512); 3 bufs of 1024 or 2 bufs of 1536 both fill 6 banks.
    sw, sbufs = (1024, 3) if slab2 else (1536, 2)

    with tile.TileContext(nc) as tc:
        import contextlib

        ctx = contextlib.ExitStack()
        with ctx:
            wpool = ctx.enter_context(tc.tile_pool(name="wpool", bufs=1))
            mpool = ctx.enter_context(tc.tile_pool(name="mpool", bufs=1))
            psum = ctx.enter_context(tc.tile_pool(name="psum", bufs=1, space="PSUM"))
            opool = ctx.enter_context(tc.tile_pool(name="opool", bufs=4))

            # ---- persistent tiles ------------------------------------------
            # x and the weights are double-banked (A/B): each body half loads
            # the NEXT half's bank up front, so staging DMAs fully overlap
            # this half's compute instead of stalling the iteration boundary.
            xrs = [mpool.tile([P, KC, N], BF16, name=f"xr{i}") for i in range(2)]
            wqrs = [wpool.tile([P, KC, GC], BF16, name=f"wqr{i}") for i in range(2)]
            wkrs = [wpool.tile([P, KC, GC], BF16, name=f"wkr{i}") for i in range(2)]
            wvrs = [wpool.tile([P, KC, GC], BF16, name=f"wvr{i}") for i in range(2)]
            wors = [wpool.tile([P, GC // P, C], BF16, name=f"wor{i}") for i in range(2)]
            qr = mpool.tile([P, 2, N], BF16)  # Q^T; dim1 = head pair (mch)
            kr = mpool.tile([P, 2, N], BF16)
            # va blocks: even heads [V | ones], odd heads [ones | V] so the
            # attention output lands on the partition half matching the
            # head's slot in `an` (the ones columns give the softmax row
            # sums for free: PE matmul cost only depends on the moving dim).
            va = mpool.tile([P, MC, HPG, P], BF16)
            an = mpool.tile([P, 2, N], BF16)  # normalized attn^T

            def prologue():
                nc.gpsimd.memset(va[:, :, 0::2, HS:P], 1.0)
                nc.gpsimd.memset(va[:, :, 1::2, 0:HS], 1.0)

            # ---- input loads (2 DGE queues, ACT kept free) -----------------
            def stage_all(bank):
                x3 = xT.rearrange("(c p) n -> p c n", p=P)
                nc.sync.dma_start(
                    out=wkrs[bank][:], in_=wkT.rearrange("(c p) m -> p c m", p=P)
                )
                nc.gpsimd.dma_start(
                    out=wqrs[bank][:], in_=wqT.rearrange("(c p) m -> p c m", p=P)
                )
                engs = [nc.sync, nc.gpsimd]
                for j in range(4):
                    engs[j % 2].dma_start(
                        out=xrs[bank][:, 2 * j : 2 * j + 2, :],
                        in_=x3[:, 2 * j : 2 * j + 2, :],
                    )
                nc.gpsimd.dma_start(
                    out=wvrs[bank][:], in_=wvT.rearrange("(c p) m -> p c m", p=P)
                )
                nc.sync.dma_start(
                    out=wors[bank][:], in_=woT.rearrange("(c p) o -> p c o", p=P)
                )

            # ---- QKV projections (single 512-query/key chunk per call) -----
            def qk_proj_nch(bank, w_rs, dst, mch, nch):
                w_r = w_rs[bank]
                acc = psum.tile([P, sw], F32, tag="s", bufs=sbufs, name="acc")
                for k in range(KC):
                    nc.tensor.matmul(
                        acc[:, 0:512],
                        w_r[:, k, mch * P : (mch + 1) * P],
                        xrs[bank][:, k, nch * 512 : (nch + 1) * 512],
                        start=(k == 0),
                        stop=(k == KC - 1),
                    )
                nc.vector.tensor_copy(
                    dst[:, mch, nch * 512 : (nch + 1) * 512], acc[:, 0:512]
                )

            def v_proj_quarter(bank, mq):
                for m in range(4 * mq, 4 * mq + 4):
                    vacc = psum.tile([P, sw], F32, tag="s", bufs=sbufs, name="vacc")
                    for k in range(KC):
                        nc.tensor.matmul(
                            vacc[:, 0:GC],
                            xrs[bank][:, k, m * P : (m + 1) * P],
                            wvrs[bank][:, k, :],
                            start=(k == 0),
                            stop=(k == KC - 1),
                        )
                    vh = vacc[:, 0:GC].rearrange("p (h e) -> p h e", h=HPG)
                    nc.vector.tensor_copy(va[:, m, 0::2, 0:HS], vh[:, 0::2, :])
                    nc.vector.tensor_copy(va[:, m, 1::2, HS:P], vh[:, 1::2, :])

            # ---- attention pass A: scores + exp + inline AV ----------------
            # Head pair slabs (par 0/1, bases 0/64) are emitted back-to-back:
            # distinct PE row groups execute concurrently.  exp writes one
            # [128,1024] p-ring tile per key chunk m; AV for chunk m-2 is
            # interleaved right after (its exp long done, so the in-order PE
            # never stalls on ACT).  Queued work (projection chunks, the
            # previous window's output slices) drains fast in the first
            # tiles -- o_ps slices MUST be emitted before av(0) takes the
            # att banks or the ring order deadlocks -- then slowly.
            def pass_a(hp, q, pending, bank):
                qsl = slice(q * 512, (q + 1) * 512)
                att = {}
                pts = [None] * MC

                def av(m):
                    if "a0" not in att:
                        att["a0"] = psum.tile(
                            [P, 512], F32, tag="att0", bufs=1, name="att0"
                        )
                        att["a1"] = psum.tile(
                            [P, 512], F32, tag="att1", bufs=1, name="att1"
                        )
                    for par, at in ((0, att["a0"]), (1, att["a1"])):
                        nc.tensor.matmul(
                            at[:],
                            va[:, m, 2 * hp + par, :],
                            pts[m][:, par * 512 : (par + 1) * 512],
                            start=(m == 0),
                            stop=(m == MC - 1),
                        )

                for m in range(MC):
                    st = psum.tile([P, sw], F32, tag="s", bufs=sbufs, name="st")
                    for par in range(2):
                        off = 0 if unpair else 64 * par  # unpair: timing probe
                        nc.tensor.matmul(
                            st[:, par * 512 : (par + 1) * 512],
                            kr[off : off + 64, hp, m * P : (m + 1) * P],
                            qr[off : off + 64, hp, qsl],
                            start=True,
                            stop=True,
                        )
                    pts[m] = mpool.tile([P, 1024], BF16, tag="pw", bufs=4, name="pt")
                    nc.scalar.activation(
                        pts[m][:], st[:, 0:1024], Exp, scale=SCALE
                    )
                    for _ in range(4 if m < 3 else pb_rate):
                        if pending:
                            pending.popleft()()
                    if m >= 2:
                        av(m - 2)
                av(MC - 2)
                av(MC - 1)

                # normalize.  Even head: attn rows 0:64, rowsum 64:128; odd
                # head flipped (va block order).  The custom recip uop only
                # works at base partition 0; cross-partition moves go
                # through SBUF->SBUF DMA.
                au0 = mpool.tile([P, 512], F32, tag="au", bufs=4, name="au0")
                au1 = mpool.tile([P, 512], F32, tag="au", bufs=4, name="au1")
                rr0 = mpool.tile([P, 512], F32, tag="rr", bufs=4, name="rr0")
                rr1 = mpool.tile([P, 512], F32, tag="rr", bufs=4, name="rr1")
                nc.vector.tensor_copy(au0[:], att["a0"][:])
                nc.vector.tensor_copy(au1[:], att["a1"][:])
                nc.sync.dma_start(out=rr0[0:64, :], in_=au0[64:128, :])
                nc.vector.reciprocal_approx_fast(rr0[0:64, :], rr0[0:64, :])
                nc.vector.tensor_mul(an[0:64, hp, qsl], au0[0:64, :], rr0[0:64, :])
                nc.vector.reciprocal_approx_fast(rr1[0:64, :], au1[0:64, :])
                nc.gpsimd.dma_start(out=rr1[64:128, :], in_=rr1[0:64, :])
                nc.vector.tensor_mul(
                    an[64:128, hp, qsl], au1[64:128, :], rr1[64:128, :]
                )

            # ---- output projection, one 128-channel slice at a time --------
            def out_step(q, och, bank):
                def f():
                    o_ps = psum.tile(
                        [P, 512], F32, tag="att0" if och % 2 == 0 else "att1",
                        bufs=1, name="o_ps",
                    )
                    for c in range(GC // P):
                        nc.tensor.matmul(
                            o_ps[:],
                            wors[bank][:, c, och * P : (och + 1) * P],
                            an[:, c, q * 512 : (q + 1) * 512],
                            start=(c == 0),
                            stop=(c == GC // P - 1),
                        )
                    o_sb = opool.tile([P, 512], F32, name="o_sb")
                    nc.vector.tensor_copy(o_sb[:], o_ps[:])
                    eng = nc.sync if och % 2 == 0 else nc.gpsimd
                    eng.dma_start(
                        out=outT[och * P : (och + 1) * P, q * 512 : (q + 1) * 512],
                        in_=o_sb,
                    )

                return f

            # ---- body half: one full MHSA pass on input bank `bank` --------
            # Window (hp, q) only reads qr chunk nch=q, so Q chunks are
            # computed one window ahead; K chunks for pair 1 spread over the
            # pair-0 windows.  `pending` persists across halves: the last
            # window's output slices drain in the next half's first tiles.
            # Only k/q chunk nch=0 is emitted ahead of the first window; the
            # remaining startup chunks pipeline through the pending queue
            # (popped at tile j, needed at tile 4j).
            def body(bank, pending, stage_next):
                if stage_next is not None:
                    stage_all(stage_next)
                qk_proj_nch(bank, wkrs, kr, 0, 0)
                qk_proj_nch(bank, wqrs, qr, 0, 0)

                def qk(w_rs, dst, mch, nch):
                    return lambda: qk_proj_nch(bank, w_rs, dst, mch, nch)

                def vq(mq):
                    return lambda: v_proj_quarter(bank, mq)

                filler = {
                    (0, 0): [
                        vq(0), qk(wkrs, kr, 0, 1), vq(1), qk(wkrs, kr, 0, 2),
                        vq(2), qk(wkrs, kr, 0, 3), vq(3), qk(wqrs, qr, 0, 1),
                    ],
                    (0, 1): [qk(wkrs, kr, 1, 0), qk(wkrs, kr, 1, 1), qk(wqrs, qr, 0, 2)],
                    (0, 2): [qk(wkrs, kr, 1, 2), qk(wkrs, kr, 1, 3), qk(wqrs, qr, 0, 3)],
                    (0, 3): [qk(wqrs, qr, 1, 0), qk(wqrs, qr, 1, 1)],
                    (1, 0): [qk(wqrs, qr, 1, 2)],
                    (1, 1): [qk(wqrs, qr, 1, 3)],
                }
                for hp in range(2):
                    for q in range(4):
                        pending.extend(filler.get((hp, q), []))
                        pass_a(hp, q, pending, bank)
                        if hp == 1:
                            pending.extend(
                                out_step(q, och, bank) for och in range(C // P)
                            )

            def flush(pending):
                while pending:
                    pending.popleft()()

            pending = deque()
            if loop_n > 1:
                stage_all(0)
                prologue()
                ET = mybir.EngineType
                half = (loop_n - 1) // 2
                odd = (loop_n - 1) % 2
                if half > 0:
                    with tc.For_i(
                        0,
                        half,
                        1,
                        hint_engines=(ET.PE, ET.Activation, ET.DVE, ET.SP),
                    ):
                        body(0, pending, stage_next=1)
                        body(1, pending, stage_next=0)
                        # queue-neutral loop body: half B's tail drains here
                        # (half A's tail already drained inside half B)
                        flush(pending)
                body(0, pending, stage_next=1 if odd else None)
                if odd:
                    body(1, pending, stage_next=None)
                flush(pending)
            else:
                stage_all(0)
                prologue()
                body(0, pending, stage_next=None)
                flush(pending)

    nc.compile()
    return nc


def shard_inputs(x, w_qkv, w_out):
    """Host-side shard prep. Returns in_maps for cores 0..7 (core = b*4+g).

    All inputs ship as bf16 (the PE consumes bf16 directly); accumulation
    on chip is fp32 and the output returns fp32."""
    import ml_dtypes

    bf16 = ml_dtypes.bfloat16
    # w_qkv row d = c_idx*3 + t  (t: 0=q, 1=k, 2=v)  [stride-3 interleave]
    wr = np.ascontiguousarray(w_qkv.reshape(C, 3, C))
    in_maps = []
    for b in range(B):
        xTb = np.ascontiguousarray(x[b].T.astype(bf16))
        for g in range(G):
            sl = slice(g * GC, (g + 1) * GC)
            in_maps.append(
                {
                    "xT": xTb,
                    "wqT": np.ascontiguousarray(wr[sl, 0, :].T.astype(bf16)),
                    "wkT": np.ascontiguousarray(wr[sl, 1, :].T.astype(bf16)),
                    "wvT": np.ascontiguousarray(wr[sl, 2, :].T.astype(bf16)),
                    "woT": np.ascontiguousarray(w_out[:, sl].T.astype(bf16)),
                }
            )
    return in_maps


def kernel(x, w_qkv, w_out, b_out):
    x = np.asarray(x, dtype=np.float32)
    w_qkv = np.asarray(w_qkv, dtype=np.float32)
    w_out = np.asarray(w_out, dtype=np.float32)
    b_out = np.asarray(b_out, dtype=np.float32)

    if "nc" not in _CACHED:
        _CACHED["nc"] = build_bass()
    nc = _CACHED["nc"]

    in_maps = shard_inputs(x, w_qkv, w_out)
    res = run_bass_kernel_spmd(nc, in_maps, core_ids=list(range(8)))

    out = np.empty((B, N, C), dtype=np.float32)
    for b in range(B):
        acc = res.results[b * G + 0]["outT"].astype(np.float32)
        for g in range(1, G):
            acc = acc + res.results[b * G + g]["outT"]
        out[b] = acc.T + b_out
    return out


if __name__ == "__main__":
    rng = np.random.default_rng(0)
    x = rng.standard_normal((B, N, C), dtype=np.float32)
    w_qkv = rng.standard_normal((3 * C, C), dtype=np.float32) * C**-0.5
    w_out = rng.standard_normal((C, C), dtype=np.float32) * C**-0.5
    b_out = np.zeros((C,), dtype=np.float32)
    got = kernel(x, w_qkv, w_out, b_out)
    print("kernel ran, output shape", got.shape)
